# revision 68
# baseline (speedup 1.0000x reference)
"""Trainium2 Bass kernel for nn_EquiPINE (pooling).

Math (per branch):
    W_g = (U @ A).reshape(M, L); w = W_g @ P  -> [M]
    g = sigmoid(x[...,None] * w + V)          -> [B, N, D, M]
    out = sum_n max_d g                       -> [B, M]

Key restructuring: sigmoid is monotonic, so
    max_d sigmoid(x*w + V) = sigmoid(max_d(x*w) + V)
and max_d(x[b,n,d]*w[m]) = w_pos[m]*xmax[b,n] + w_neg[m]*xmin[b,n]
(with w_pos = max(w,0), w_neg = min(w,0)).  w is split into bf16 hi+lo
(its rounding error would be systematic across the n-sum); xmax/xmin
go to plain bf16 (their rounding is random across n and averages out -
verified 4e-7 end-to-end).  The whole [B,N,D,M] intermediate collapses
into one K=4 bf16 matmul -> PSUM [128, N] per (batch, branch), then one
ACT sigmoid (+per-partition bias V, accumulate-over-free) producing the
pooled z column directly.

Sharding: data-parallel over batch; 8 batches per core on 8 cores.
Params are tiny and replicated; host precomputes w hi/lo rows, W_h^T,
C_w^T and packs all f32 params into one DMA.

DMA budget notes: every hwdge dma_start costs ~625ns of issuing-engine
time and all of an engine's transfers share one hardware queue
(~90-200 GB/s), so transfers are split across the sync/scalar/gpsimd
queues and the scalar engine is kept DMA-free once the sigmoid phase
starts.
"""

import numpy as np

import concourse.bass as bass
import concourse.bacc as bacc
import concourse.tile as tile
from concourse import mybir
from concourse.bass_utils import run_bass_kernel_spmd

NCORES = 8
B = 64
B_LOC = B // NCORES  # 8 batches per core
N = 1024
D = 16
M = 128
L = 32
H = 256
O = 128

F32 = mybir.dt.float32
BF16 = mybir.dt.bfloat16
AF = mybir.ActivationFunctionType
ALU = mybir.AluOpType
AX = mybir.AxisListType

# Transfer time ~ max(n_descriptors * 20ns, bytes / 300GB/s), one
# descriptor per partition line.  x0 ships alone on the sync queue
# (critical path); x1 + every parameter ride one DMA on the scalar
# queue: x1pack column layout after x1's 1024 is
#   wht0 wht1 cw0 cw1 v0 v1 | lhs(bf16 pair-packed, rows 0..3) | cb(row 0)
PK_BASE = 1024
PK_WHT0 = PK_BASE + 0
PK_WHT1 = PK_BASE + 256
PK_CW0 = PK_BASE + 512
PK_CW1 = PK_BASE + 640
PK_V0 = PK_BASE + 768
PK_V1 = PK_BASE + 769
PK_LHS = PK_BASE + 770  # [4, 128] f32 words = [4, 256] bf16
PK_CB = PK_BASE + 898  # [1, 128]
X1P_COLS = PK_BASE + 1026


def _emit(tc, io):
    nc = tc.nc
    with (
        tc.tile_pool(name="const", bufs=1) as cpool,
        tc.tile_pool(name="xp", bufs=1) as xpool,
        tc.tile_pool(name="stat", bufs=1) as spool,
        tc.tile_pool(name="sig", bufs=2) as sigpool,
        tc.tile_pool(name="ps", bufs=3, space="PSUM") as pspool,
        tc.tile_pool(name="psmlp", bufs=2, space="PSUM") as mlppool,
    ):
        # ---- loads: x0 halves split across the sync+scalar queues for
        # parallel transfer; x1 on the gpsimd SWDGE queue; params early
        # on scalar (all scalar DMA gen finishes before sigmoids start).
        x0t = xpool.tile([128, N], F32, tag="x0t", name="x0t")
        nc.sync.dma_start(x0t[:], io["x0"])
        # x1 + the big MLP weights ride the otherwise-idle gpsimd SWDGE
        # queue; the small early-needed params (lhs, cb, V) get their own
        # tiny DMAs on scalar so nothing big sits ahead of the row
        # shuffles in the scalar queue.
        x1p = xpool.tile([128, X1P_COLS], F32, tag="x1p", name="x1p")
        nc.gpsimd.dma_start(x1p[:], io["x1p"])
        xt0 = x0t[:]
        xt1 = x1p[:, 0:N]
        pack = x1p
        # lhs rows 0-3: wph wpl wnh wnl per branch; rows 64-65 replicate
        # branch-0's (wnh, wnl) so unit b0 can run as two K=2 matmuls
        # with base partitions 0 and 64 (xm part fires before the xn
        # rows land, cutting ~1us off the first-sigmoid time)
        lhs_t = cpool.tile([66, 2 * M], BF16, tag="lhs")
        nc.scalar.dma_start(lhs_t[:], io["lhs"])
        lhs = lhs_t[:]
        cbv = cpool.tile([1, O], F32, tag="cbv")
        nc.scalar.dma_start(cbv[:], io["cb"])
        cb_t = cbv[:]
        vv = cpool.tile([128, 2], F32, tag="vv")
        nc.scalar.dma_start(vv[:], io["vv"])
        ones_t = cpool.tile([1, B_LOC], F32, tag="ones")
        nc.vector.memset(ones_t[:], 1.0)
        # dummy activation early in the scalar stream hoists the
        # conservative mid-block ACT_TABLE_LOAD out of the critical path
        dummy = spool.tile([1, 1], F32, tag="dummy", name="dummy")
        nc.scalar.activation(dummy[:], ones_t[0:1, 0:1], AF.Sigmoid)

        # z columns per branch: [M, B_LOC]
        z_t = [
            cpool.tile([M, B_LOC], F32, tag=f"z{br}", name=f"z{br}")
            for br in range(2)
        ]

        # ---- per x-tensor: d-reduce (max/min), bf16 cast, row shuffle ----
        # x shard viewed as [128, 1024]: partition p = b*16 + n//64,
        # free f = (n%64)*16 + d.
        xt = [xt0, xt1]
        from concourse.bass import _add_dep_helper

        r_t = []
        last_cast = None
        for xi in range(2):
            x3 = xt[xi].rearrange("p (c d) -> p c d", d=D)
            xmax = spool.tile([128, 64], F32, tag=f"xmax{xi}", name="xmax")
            xmin = spool.tile([128, 64], F32, tag=f"xmin{xi}", name="xmin")
            comb = spool.tile([128, 128], BF16, tag=f"comb{xi}", name="comb")
            # DVE stream: red_max, cast_max, red_min, cast_min - each cast
            # unblocks its row-shuffle DMAs as early as possible
            ops = []
            ops.append(nc.vector.tensor_reduce(xmax[:], x3, axis=AX.X, op=ALU.max))
            ops.append(nc.vector.tensor_copy(comb[:, 0:64], xmax[:]))
            ops.append(nc.vector.tensor_reduce(xmin[:], x3, axis=AX.X, op=ALU.min))
            ops.append(nc.vector.tensor_copy(comb[:, 64:128], xmin[:]))
            # cast_max must precede red_min in the static DVE order so the
            # xm row shuffles launch while red_min still runs
            _add_dep_helper(
                ops[2].ins, ops[1].ins, sync=False, reason="cast before next reduce"
            )
            if last_cast is not None:
                # keep x1's DVE work behind x0's casts in the static DVE
                # stream (the scheduler otherwise interleaves them and
                # head-of-line-blocks x0's row shuffle for ~2us)
                for op in ops:
                    _add_dep_helper(
                        op.ins, last_cast.ins, sync=False, reason="x0 chain first"
                    )
            last_cast = ops[-1]
            if xi == 0:
                r_ops0 = ops[0]
            # R rows = xm xm xn xn; free = b*1024 + c*64 + j (n = c*64+j).
            # Row transfers are descriptor-bound (~20ns x 16 descriptors
            # per batch), so x0's rows are split into an early b0-2 chunk
            # (lets the matmul/sigmoid phase start ~2us sooner) + bulk.
            rt = spool.tile(
                [66 if xi == 0 else 4, B_LOC * N], BF16, tag=f"r{xi}", name="rt"
            )
            if xi == 0:
                chunks = [(0, 2), (2, B_LOC)]
                engs = [nc.sync, nc.scalar]
            else:
                chunks = [(0, B_LOC)]
                engs = [nc.sync]
            for ci, (b0, b1) in enumerate(chunks):
                for row, src in enumerate([0, 0, 1, 1]):
                    engs[row % len(engs)].dma_start(
                        rt[
                            row : row + 1, b0 * N : b1 * N
                        ].rearrange("p (b c j) -> p b c j", c=16, j=64),
                        comb[b0 * 16 : b1 * 16, src * 64 : (src + 1) * 64],
                    )
                if xi == 0 and ci == 0:
                    # xn rows for batch 0 replicated at partitions 64-65
                    for k in range(2):
                        engs[k].dma_start(
                            rt[64 + k : 65 + k, 0:N].rearrange(
                                "p (b c j) -> p b c j", c=16, j=64
                            ),
                            comb[0:16, 64:128],
                        )
            r_t.append(rt)

        # ---- branch core: K=4 matmul + fused sigmoid/bias/accum ----
        # lhs rows: wph wpl wnh wnl pair with rt rows: xm xm xn xn
        def unit(br, b):
            lt = lhs[0:4, br * M : (br + 1) * M]
            vt = vv[:, br : br + 1]
            ps = pspool.tile([M, N], F32, tag="s", name="ps")
            rhs = r_t[br][0:4, b * N : (b + 1) * N]
            nc.tensor.matmul(ps[:, 0:512], lt, rhs[:, 0:512], start=True, stop=True)
            nc.tensor.matmul(
                ps[:, 512:1024], lt, rhs[:, 512:1024], start=True, stop=True
            )
            sg = sigpool.tile([M, N], F32, tag="sg", name="sg")
            nc.scalar.activation(
                sg[:],
                ps[:],
                AF.Sigmoid,
                bias=vt,
                accum_out=z_t[br][:, b : b + 1],
            )

        # ---- MLP head (per batch-half so half 0 hides under sigmoids):
        # h = sigmoid(W_h @ z); y = C_w @ h + C_b ----
        y_half = [
            spool.tile([4, O], F32, tag=f"ysb{h}", name=f"ysb{h}")
            for h in range(2)
        ]

        def mlp_half(half):
            bs = slice(half * 4, half * 4 + 4)
            h_t = []
            for hh in range(2):
                hp = mlppool.tile([128, 4], F32, tag="mlp", name="hp")
                nc.tensor.matmul(
                    hp[:],
                    pack[:, PK_WHT0 + hh * 128 : PK_WHT0 + (hh + 1) * 128],
                    z_t[0][:, bs],
                    start=True,
                    stop=False,
                )
                nc.tensor.matmul(
                    hp[:],
                    pack[:, PK_WHT1 + hh * 128 : PK_WHT1 + (hh + 1) * 128],
                    z_t[1][:, bs],
                    start=False,
                    stop=True,
                )
                hs = spool.tile([128, 4], F32, tag=f"hs{hh}_{half}", name="hs")
                nc.scalar.activation(hs[:], hp[:], AF.Sigmoid)
                h_t.append(hs)
            yp = mlppool.tile([4, O], F32, tag="mlp", name="yp")
            nc.tensor.matmul(
                yp[:], h_t[0][:], pack[:, PK_CW0 : PK_CW0 + O], start=True, stop=False
            )
            nc.tensor.matmul(
                yp[:], h_t[1][:], pack[:, PK_CW1 : PK_CW1 + O], start=False, stop=False
            )
            nc.tensor.matmul(yp[:], ones_t[:, 0:4], cb_t, start=False, stop=True)
            nc.vector.tensor_copy(y_half[half][:], yp[:])

        for b in range(B_LOC):
            unit(0, b)
        for b in range(B_LOC):
            unit(1, b)
            if b == 3:
                mlp_half(0)
        mlp_half(1)
        nc.sync.dma_start(io["y"][0:4, :], y_half[0][:])
        nc.scalar.dma_start(io["y"][4:8, :], y_half[1][:])


_CACHED = None


def _build():
    global _CACHED
    if _CACHED is not None:
        return _CACHED
    nc = bacc.Bacc(
        "TRN2", target_bir_lowering=False, debug=False, num_devices=NCORES
    )
    io = {}
    io["x0"] = nc.dram_tensor("x0", [128, N], F32, kind="ExternalInput").ap()
    io["x1p"] = nc.dram_tensor(
        "x1p", [128, X1P_COLS], F32, kind="ExternalInput"
    ).ap()
    io["lhs"] = nc.dram_tensor("lhs", [66, 2 * M], BF16, kind="ExternalInput").ap()
    io["cb"] = nc.dram_tensor("cb", [1, O], F32, kind="ExternalInput").ap()
    io["vv"] = nc.dram_tensor("vv", [128, 2], F32, kind="ExternalInput").ap()
    io["y"] = nc.dram_tensor("y", [B_LOC, O], F32, kind="ExternalOutput").ap()

    with tile.TileContext(nc) as tc:
        _emit(tc, io)
    nc.compile()
    _CACHED = nc
    return nc


def _prep_params(inputs):
    import ml_dtypes

    f = np.float32
    bf = ml_dtypes.bfloat16

    def branch_lhs(P, U, A):
        W_g = (U @ A).reshape(M, L).astype(np.float64)
        w = (W_g @ P.astype(np.float64))[:, 0]
        rows = []
        for part in (np.maximum(w, 0.0), np.minimum(w, 0.0)):
            hi = part.astype(f).astype(bf)
            lo = (part.astype(f) - hi.astype(f)).astype(bf)
            rows += [hi, lo]
        # rows: wph wpl wnh wnl (pair with R's xm xm xn xn)
        return np.stack(rows).astype(bf)

    pack = np.zeros((128, X1P_COLS - PK_BASE), dtype=f)
    pack[:, PK_WHT0 - PK_BASE : PK_WHT0 - PK_BASE + 256] = inputs["W_h"].T[0:128, :]
    pack[:, PK_WHT1 - PK_BASE : PK_WHT1 - PK_BASE + 256] = inputs["W_h"].T[128:256, :]
    pack[:, PK_CW0 - PK_BASE : PK_CW0 - PK_BASE + O] = inputs["C_w"].T[0:128, :]
    pack[:, PK_CW1 - PK_BASE : PK_CW1 - PK_BASE + O] = inputs["C_w"].T[128:256, :]
    pack[:, PK_V0 - PK_BASE] = inputs["V0"].astype(f)
    pack[:, PK_V1 - PK_BASE] = inputs["V1"].astype(f)

    lhs = np.concatenate(
        [
            branch_lhs(inputs["P0"], inputs["U0"], inputs["A0"]),
            branch_lhs(inputs["P1"], inputs["U1"], inputs["A1"]),
        ],
        axis=1,
    )  # [4, 256] bf16
    lhs_full = np.zeros((66, 2 * M), dtype=bf)
    lhs_full[0:4, :] = lhs
    lhs_full[64:66, 0:M] = lhs[2:4, 0:M]  # branch-0 (wnh, wnl) replica

    vv = np.stack([inputs["V0"].astype(f), inputs["V1"].astype(f)], axis=1)
    return {
        "pack": pack,
        "lhs": np.ascontiguousarray(lhs_full),
        "cb": np.ascontiguousarray(inputs["C_b"].reshape(1, O), dtype=f),
        "vv": np.ascontiguousarray(vv),
    }


def run(inputs, trace=False, **kw):
    nc = _build()
    params = _prep_params(inputs)
    x0 = np.ascontiguousarray(inputs["x0"], dtype=np.float32)
    x1 = np.ascontiguousarray(inputs["x1"], dtype=np.float32)
    in_maps = []
    for c in range(NCORES):
        m = {k: v for k, v in params.items() if k != "pack"}
        m["x0"] = x0[c * B_LOC : (c + 1) * B_LOC].reshape(128, N)
        m["x1p"] = np.concatenate(
            [x1[c * B_LOC : (c + 1) * B_LOC].reshape(128, N), params["pack"]],
            axis=1,
        )
        in_maps.append(m)
    res = run_bass_kernel_spmd(nc, in_maps, list(range(NCORES)), trace=trace, **kw)
    y = np.concatenate([res.results[c]["y"] for c in range(NCORES)], axis=0)
    return y, res


def kernel(**inputs):
    y, _ = run(inputs, trace=False)
    return y


# revision 69
# speedup vs baseline: 1.0871x; 1.0871x over previous
"""Trainium2 Bass kernel for nn_EquiPINE (pooling).

Math (per branch):
    W_g = (U @ A).reshape(M, L); w = W_g @ P  -> [M]
    g = sigmoid(x[...,None] * w + V)          -> [B, N, D, M]
    out = sum_n max_d g                       -> [B, M]

Key restructuring: sigmoid is monotonic, so
    max_d sigmoid(x*w + V) = sigmoid(max_d(x*w) + V)
and max_d(x[b,n,d]*w[m]) = w_pos[m]*xmax[b,n] + w_neg[m]*xmin[b,n]
(with w_pos = max(w,0), w_neg = min(w,0)).  w is split into bf16 hi+lo
(its rounding error would be systematic across the n-sum); xmax/xmin
go to plain bf16 (their rounding is random across n and averages out -
verified 4e-7 end-to-end).  The whole [B,N,D,M] intermediate collapses
into one K=4 bf16 matmul -> PSUM [128, N] per (batch, branch), then one
ACT sigmoid (+per-partition bias V, accumulate-over-free) producing the
pooled z column directly.

Sharding: data-parallel over batch; 8 batches per core on 8 cores.
Params are tiny and replicated; host precomputes w hi/lo rows, W_h^T,
C_w^T and packs all f32 params into one DMA.

DMA budget notes: every hwdge dma_start costs ~625ns of issuing-engine
time and all of an engine's transfers share one hardware queue
(~90-200 GB/s), so transfers are split across the sync/scalar/gpsimd
queues and the scalar engine is kept DMA-free once the sigmoid phase
starts.
"""

import numpy as np

import concourse.bass as bass
import concourse.bacc as bacc
import concourse.tile as tile
from concourse import mybir
from concourse.bass_utils import run_bass_kernel_spmd

NCORES = 8
B = 64
B_LOC = B // NCORES  # 8 batches per core
N = 1024
D = 16
M = 128
L = 32
H = 256
O = 128

F32 = mybir.dt.float32
BF16 = mybir.dt.bfloat16
AF = mybir.ActivationFunctionType
ALU = mybir.AluOpType
AX = mybir.AxisListType

# Transfer time ~ max(n_descriptors * 20ns, bytes / 300GB/s), one
# descriptor per partition line.  x0 ships alone on the sync queue
# (critical path); x1 + every parameter ride one DMA on the scalar
# queue: x1pack column layout after x1's 1024 is
#   wht0 wht1 cw0 cw1 v0 v1 | lhs(bf16 pair-packed, rows 0..3) | cb(row 0)
PK_BASE = 1024
PK_WHT0 = PK_BASE + 0
PK_WHT1 = PK_BASE + 256
PK_CW0 = PK_BASE + 512
PK_CW1 = PK_BASE + 640
PK_V0 = PK_BASE + 768
PK_V1 = PK_BASE + 769
PK_LHS = PK_BASE + 770  # [4, 128] f32 words = [4, 256] bf16
PK_CB = PK_BASE + 898  # [1, 128]
X1P_COLS = PK_BASE + 1026


def _emit(tc, io):
    nc = tc.nc
    with (
        tc.tile_pool(name="const", bufs=1) as cpool,
        tc.tile_pool(name="xp", bufs=1) as xpool,
        tc.tile_pool(name="stat", bufs=1) as spool,
        tc.tile_pool(name="sig", bufs=2) as sigpool,
        tc.tile_pool(name="ps", bufs=3, space="PSUM") as pspool,
        tc.tile_pool(name="psmlp", bufs=2, space="PSUM") as mlppool,
    ):
        # ---- loads: x0 halves split across the sync+scalar queues for
        # parallel transfer; x1 on the gpsimd SWDGE queue; params early
        # on scalar (all scalar DMA gen finishes before sigmoids start).
        x0t = xpool.tile([128, N], F32, tag="x0t", name="x0t")
        nc.sync.dma_start(x0t[:], io["x0"])
        # x1 + the big MLP weights ride the otherwise-idle gpsimd SWDGE
        # queue; the small early-needed params (lhs, cb, V) get their own
        # tiny DMAs on scalar so nothing big sits ahead of the row
        # shuffles in the scalar queue.
        x1p = xpool.tile([128, X1P_COLS], F32, tag="x1p", name="x1p")
        nc.gpsimd.dma_start(x1p[:], io["x1p"])
        xt0 = x0t[:]
        xt1 = x1p[:, 0:N]
        pack = x1p
        # lhs rows 0-3: wph wpl wnh wnl per branch; rows 64-65 replicate
        # branch-0's (wnh, wnl) so unit b0 can run as two K=2 matmuls
        # with base partitions 0 and 64 (xm part fires before the xn
        # rows land, cutting ~1us off the first-sigmoid time)
        lhs_t = cpool.tile([66, 2 * M], BF16, tag="lhs")
        nc.scalar.dma_start(lhs_t[:], io["lhs"])
        lhs = lhs_t[:]
        cbv = cpool.tile([1, O], F32, tag="cbv")
        nc.scalar.dma_start(cbv[:], io["cb"])
        cb_t = cbv[:]
        vv = cpool.tile([128, 2], F32, tag="vv")
        nc.scalar.dma_start(vv[:], io["vv"])
        ones_t = cpool.tile([1, B_LOC], F32, tag="ones")
        nc.vector.memset(ones_t[:], 1.0)
        # dummy activation early in the scalar stream hoists the
        # conservative mid-block ACT_TABLE_LOAD out of the critical path
        dummy = spool.tile([1, 1], F32, tag="dummy", name="dummy")
        nc.scalar.activation(dummy[:], ones_t[0:1, 0:1], AF.Sigmoid)

        # z columns per branch: [M, B_LOC]
        z_t = [
            cpool.tile([M, B_LOC], F32, tag=f"z{br}", name=f"z{br}")
            for br in range(2)
        ]

        # ---- per x-tensor: d-reduce (max/min), bf16 cast, row shuffle ----
        # x shard viewed as [128, 1024]: partition p = b*16 + n//64,
        # free f = (n%64)*16 + d.
        xt = [xt0, xt1]
        from concourse.bass import _add_dep_helper

        r_t = []
        last_cast = None
        for xi in range(2):
            x3 = xt[xi].rearrange("p (c d) -> p c d", d=D)
            xmax = spool.tile([128, 64], F32, tag=f"xmax{xi}", name="xmax")
            xmin = spool.tile([128, 64], F32, tag=f"xmin{xi}", name="xmin")
            comb = spool.tile([128, 128], BF16, tag=f"comb{xi}", name="comb")
            # DVE stream: red_max, cast_max, red_min, cast_min - each cast
            # unblocks its row-shuffle DMAs as early as possible
            ops = []
            ops.append(nc.vector.tensor_reduce(xmax[:], x3, axis=AX.X, op=ALU.max))
            ops.append(nc.vector.tensor_copy(comb[:, 0:64], xmax[:]))
            ops.append(nc.vector.tensor_reduce(xmin[:], x3, axis=AX.X, op=ALU.min))
            ops.append(nc.vector.tensor_copy(comb[:, 64:128], xmin[:]))
            # cast_max must precede red_min in the static DVE order so the
            # xm row shuffles launch while red_min still runs
            _add_dep_helper(
                ops[2].ins, ops[1].ins, sync=False, reason="cast before next reduce"
            )
            if last_cast is not None:
                # keep x1's DVE work behind x0's casts in the static DVE
                # stream (the scheduler otherwise interleaves them and
                # head-of-line-blocks x0's row shuffle for ~2us)
                for op in ops:
                    _add_dep_helper(
                        op.ins, last_cast.ins, sync=False, reason="x0 chain first"
                    )
            last_cast = ops[-1]
            if xi == 0:
                r_ops0 = ops[0]
            # R rows = xm xm xn xn; free = b*1024 + c*64 + j (n = c*64+j).
            # Row transfers are descriptor-bound (~20ns x 16 descriptors
            # per batch), so x0's rows are split into an early b0-2 chunk
            # (lets the matmul/sigmoid phase start ~2us sooner) + bulk.
            rt = spool.tile(
                [66 if xi == 0 else 4, B_LOC * N], BF16, tag=f"r{xi}", name="rt"
            )
            if xi == 0:
                chunks = [(0, 3), (3, B_LOC)]
                engs = [nc.sync, nc.scalar]
            else:
                chunks = [(0, B_LOC)]
                engs = [nc.sync]
            for ci, (b0, b1) in enumerate(chunks):
                for row, src in enumerate([0, 0, 1, 1]):
                    engs[row % len(engs)].dma_start(
                        rt[
                            row : row + 1, b0 * N : b1 * N
                        ].rearrange("p (b c j) -> p b c j", c=16, j=64),
                        comb[b0 * 16 : b1 * 16, src * 64 : (src + 1) * 64],
                    )
                if xi == 0 and ci == 0:
                    # xn rows for batch 0 replicated at partitions 64-65
                    for k in range(2):
                        engs[k].dma_start(
                            rt[64 + k : 65 + k, 0:N].rearrange(
                                "p (b c j) -> p b c j", c=16, j=64
                            ),
                            comb[0:16, 64:128],
                        )
            r_t.append(rt)

        # ---- branch core: K=4 matmul + fused sigmoid/bias/accum ----
        # lhs rows: wph wpl wnh wnl pair with rt rows: xm xm xn xn
        def unit(br, b):
            lt = lhs[0:4, br * M : (br + 1) * M]
            vt = vv[:, br : br + 1]
            ps = pspool.tile([M, N], F32, tag="s", name="ps")
            rhs = r_t[br][0:4, b * N : (b + 1) * N]
            nc.tensor.matmul(ps[:, 0:512], lt, rhs[:, 0:512], start=True, stop=True)
            nc.tensor.matmul(
                ps[:, 512:1024], lt, rhs[:, 512:1024], start=True, stop=True
            )
            sg = sigpool.tile([M, N], F32, tag="sg", name="sg")
            nc.scalar.activation(
                sg[:],
                ps[:],
                AF.Sigmoid,
                bias=vt,
                accum_out=z_t[br][:, b : b + 1],
            )

        # ---- MLP head (per batch-half so half 0 hides under sigmoids):
        # h = sigmoid(W_h @ z); y = C_w @ h + C_b ----
        y_half = [
            spool.tile([4, O], F32, tag=f"ysb{h}", name=f"ysb{h}")
            for h in range(2)
        ]

        def mlp_half(half):
            bs = slice(half * 4, half * 4 + 4)
            h_t = []
            for hh in range(2):
                hp = mlppool.tile([128, 4], F32, tag="mlp", name="hp")
                nc.tensor.matmul(
                    hp[:],
                    pack[:, PK_WHT0 + hh * 128 : PK_WHT0 + (hh + 1) * 128],
                    z_t[0][:, bs],
                    start=True,
                    stop=False,
                )
                nc.tensor.matmul(
                    hp[:],
                    pack[:, PK_WHT1 + hh * 128 : PK_WHT1 + (hh + 1) * 128],
                    z_t[1][:, bs],
                    start=False,
                    stop=True,
                )
                hs = spool.tile([128, 4], F32, tag=f"hs{hh}_{half}", name="hs")
                nc.scalar.activation(hs[:], hp[:], AF.Sigmoid)
                h_t.append(hs)
            yp = mlppool.tile([4, O], F32, tag="mlp", name="yp")
            nc.tensor.matmul(
                yp[:], h_t[0][:], pack[:, PK_CW0 : PK_CW0 + O], start=True, stop=False
            )
            nc.tensor.matmul(
                yp[:], h_t[1][:], pack[:, PK_CW1 : PK_CW1 + O], start=False, stop=False
            )
            nc.tensor.matmul(yp[:], ones_t[:, 0:4], cb_t, start=False, stop=True)
            nc.vector.tensor_copy(y_half[half][:], yp[:])

        for b in range(B_LOC):
            unit(0, b)
        for b in range(B_LOC):
            unit(1, b)
            if b == 3:
                mlp_half(0)
        mlp_half(1)
        nc.sync.dma_start(io["y"][0:4, :], y_half[0][:])
        nc.scalar.dma_start(io["y"][4:8, :], y_half[1][:])


_CACHED = None


def _build():
    global _CACHED
    if _CACHED is not None:
        return _CACHED
    nc = bacc.Bacc(
        "TRN2", target_bir_lowering=False, debug=False, num_devices=NCORES
    )
    io = {}
    io["x0"] = nc.dram_tensor("x0", [128, N], F32, kind="ExternalInput").ap()
    io["x1p"] = nc.dram_tensor(
        "x1p", [128, X1P_COLS], F32, kind="ExternalInput"
    ).ap()
    io["lhs"] = nc.dram_tensor("lhs", [66, 2 * M], BF16, kind="ExternalInput").ap()
    io["cb"] = nc.dram_tensor("cb", [1, O], F32, kind="ExternalInput").ap()
    io["vv"] = nc.dram_tensor("vv", [128, 2], F32, kind="ExternalInput").ap()
    io["y"] = nc.dram_tensor("y", [B_LOC, O], F32, kind="ExternalOutput").ap()

    with tile.TileContext(nc) as tc:
        _emit(tc, io)
    nc.compile()
    _CACHED = nc
    return nc


def _prep_params(inputs):
    import ml_dtypes

    f = np.float32
    bf = ml_dtypes.bfloat16

    def branch_lhs(P, U, A):
        W_g = (U @ A).reshape(M, L).astype(np.float64)
        w = (W_g @ P.astype(np.float64))[:, 0]
        rows = []
        for part in (np.maximum(w, 0.0), np.minimum(w, 0.0)):
            hi = part.astype(f).astype(bf)
            lo = (part.astype(f) - hi.astype(f)).astype(bf)
            rows += [hi, lo]
        # rows: wph wpl wnh wnl (pair with R's xm xm xn xn)
        return np.stack(rows).astype(bf)

    pack = np.zeros((128, X1P_COLS - PK_BASE), dtype=f)
    pack[:, PK_WHT0 - PK_BASE : PK_WHT0 - PK_BASE + 256] = inputs["W_h"].T[0:128, :]
    pack[:, PK_WHT1 - PK_BASE : PK_WHT1 - PK_BASE + 256] = inputs["W_h"].T[128:256, :]
    pack[:, PK_CW0 - PK_BASE : PK_CW0 - PK_BASE + O] = inputs["C_w"].T[0:128, :]
    pack[:, PK_CW1 - PK_BASE : PK_CW1 - PK_BASE + O] = inputs["C_w"].T[128:256, :]
    pack[:, PK_V0 - PK_BASE] = inputs["V0"].astype(f)
    pack[:, PK_V1 - PK_BASE] = inputs["V1"].astype(f)

    lhs = np.concatenate(
        [
            branch_lhs(inputs["P0"], inputs["U0"], inputs["A0"]),
            branch_lhs(inputs["P1"], inputs["U1"], inputs["A1"]),
        ],
        axis=1,
    )  # [4, 256] bf16
    lhs_full = np.zeros((66, 2 * M), dtype=bf)
    lhs_full[0:4, :] = lhs
    lhs_full[64:66, 0:M] = lhs[2:4, 0:M]  # branch-0 (wnh, wnl) replica

    vv = np.stack([inputs["V0"].astype(f), inputs["V1"].astype(f)], axis=1)
    return {
        "pack": pack,
        "lhs": np.ascontiguousarray(lhs_full),
        "cb": np.ascontiguousarray(inputs["C_b"].reshape(1, O), dtype=f),
        "vv": np.ascontiguousarray(vv),
    }


def run(inputs, trace=False, **kw):
    nc = _build()
    params = _prep_params(inputs)
    x0 = np.ascontiguousarray(inputs["x0"], dtype=np.float32)
    x1 = np.ascontiguousarray(inputs["x1"], dtype=np.float32)
    in_maps = []
    for c in range(NCORES):
        m = {k: v for k, v in params.items() if k != "pack"}
        m["x0"] = x0[c * B_LOC : (c + 1) * B_LOC].reshape(128, N)
        m["x1p"] = np.concatenate(
            [x1[c * B_LOC : (c + 1) * B_LOC].reshape(128, N), params["pack"]],
            axis=1,
        )
        in_maps.append(m)
    res = run_bass_kernel_spmd(nc, in_maps, list(range(NCORES)), trace=trace, **kw)
    y = np.concatenate([res.results[c]["y"] for c in range(NCORES)], axis=0)
    return y, res


def kernel(**inputs):
    y, _ = run(inputs, trace=False)
    return y


# revision 70
# speedup vs baseline: 1.0986x; 1.0105x over previous
"""Trainium2 Bass kernel for nn_EquiPINE (pooling).

Math (per branch):
    W_g = (U @ A).reshape(M, L); w = W_g @ P  -> [M]
    g = sigmoid(x[...,None] * w + V)          -> [B, N, D, M]
    out = sum_n max_d g                       -> [B, M]

Key restructuring: sigmoid is monotonic, so
    max_d sigmoid(x*w + V) = sigmoid(max_d(x*w) + V)
and max_d(x[b,n,d]*w[m]) = w_pos[m]*xmax[b,n] + w_neg[m]*xmin[b,n]
(with w_pos = max(w,0), w_neg = min(w,0)).  w is split into bf16 hi+lo
(its rounding error would be systematic across the n-sum); xmax/xmin
go to plain bf16 (their rounding is random across n and averages out -
verified 4e-7 end-to-end).  The whole [B,N,D,M] intermediate collapses
into one K=4 bf16 matmul -> PSUM [128, N] per (batch, branch), then one
ACT sigmoid (+per-partition bias V, accumulate-over-free) producing the
pooled z column directly.

Sharding: data-parallel over batch; 8 batches per core on 8 cores.
Params are tiny and replicated; host precomputes w hi/lo rows, W_h^T,
C_w^T and packs all f32 params into one DMA.

DMA budget notes: every hwdge dma_start costs ~625ns of issuing-engine
time and all of an engine's transfers share one hardware queue
(~90-200 GB/s), so transfers are split across the sync/scalar/gpsimd
queues and the scalar engine is kept DMA-free once the sigmoid phase
starts.
"""

import numpy as np

import concourse.bass as bass
import concourse.bacc as bacc
import concourse.tile as tile
from concourse import mybir
from concourse.bass_utils import run_bass_kernel_spmd

NCORES = 8
B = 64
B_LOC = B // NCORES  # 8 batches per core
N = 1024
D = 16
M = 128
L = 32
H = 256
O = 128

F32 = mybir.dt.float32
BF16 = mybir.dt.bfloat16
AF = mybir.ActivationFunctionType
ALU = mybir.AluOpType
AX = mybir.AxisListType

# Transfer time ~ max(n_descriptors * 20ns, bytes / 300GB/s), one
# descriptor per partition line.  x0 ships alone on the sync queue
# (critical path); x1 + every parameter ride one DMA on the scalar
# queue: x1pack column layout after x1's 1024 is
#   wht0 wht1 cw0 cw1 v0 v1 | lhs(bf16 pair-packed, rows 0..3) | cb(row 0)
PK_BASE = 1024
PK_WHT0 = PK_BASE + 0
PK_WHT1 = PK_BASE + 256
PK_CW0 = PK_BASE + 512
PK_CW1 = PK_BASE + 640
PK_V0 = PK_BASE + 768
PK_V1 = PK_BASE + 769
PK_LHS = PK_BASE + 770  # [4, 128] f32 words = [4, 256] bf16
PK_CB = PK_BASE + 898  # [1, 128]
X1P_COLS = PK_BASE + 1026


def _emit(tc, io):
    nc = tc.nc
    with (
        tc.tile_pool(name="const", bufs=1) as cpool,
        tc.tile_pool(name="xp", bufs=1) as xpool,
        tc.tile_pool(name="stat", bufs=1) as spool,
        tc.tile_pool(name="sig", bufs=2) as sigpool,
        tc.tile_pool(name="ps", bufs=3, space="PSUM") as pspool,
        tc.tile_pool(name="psmlp", bufs=2, space="PSUM") as mlppool,
    ):
        # ---- loads: x0 halves split across the sync+scalar queues for
        # parallel transfer; x1 on the gpsimd SWDGE queue; params early
        # on scalar (all scalar DMA gen finishes before sigmoids start).
        x0t = xpool.tile([128, N], F32, tag="x0t", name="x0t")
        nc.sync.dma_start(x0t[:], io["x0"])
        # x1 + the big MLP weights ride the otherwise-idle gpsimd SWDGE
        # queue; the small early-needed params (lhs, cb, V) get their own
        # tiny DMAs on scalar so nothing big sits ahead of the row
        # shuffles in the scalar queue.
        x1p = xpool.tile([128, X1P_COLS], F32, tag="x1p", name="x1p")
        nc.gpsimd.dma_start(x1p[:], io["x1p"])
        xt0 = x0t[:]
        xt1 = x1p[:, 0:N]
        pack = x1p
        # lhs rows 0-3: wph wpl wnh wnl per branch; rows 64-65 replicate
        # branch-0's (wnh, wnl) so unit b0 can run as two K=2 matmuls
        # with base partitions 0 and 64 (xm part fires before the xn
        # rows land, cutting ~1us off the first-sigmoid time)
        lhs_t = cpool.tile([66, 2 * M], BF16, tag="lhs")
        nc.scalar.dma_start(lhs_t[:], io["lhs"])
        lhs = lhs_t[:]
        cbv = cpool.tile([1, O], F32, tag="cbv")
        nc.scalar.dma_start(cbv[:], io["cb"])
        cb_t = cbv[:]
        vv = cpool.tile([128, 2], F32, tag="vv")
        nc.scalar.dma_start(vv[:], io["vv"])
        ones_t = cpool.tile([1, B_LOC], F32, tag="ones")
        nc.vector.memset(ones_t[:], 1.0)
        # dummy activation early in the scalar stream hoists the
        # conservative mid-block ACT_TABLE_LOAD out of the critical path
        dummy = spool.tile([1, 1], F32, tag="dummy", name="dummy")
        nc.scalar.activation(dummy[:], ones_t[0:1, 0:1], AF.Sigmoid)

        # z columns per branch: [M, B_LOC]
        z_t = [
            cpool.tile([M, B_LOC], F32, tag=f"z{br}", name=f"z{br}")
            for br in range(2)
        ]

        # ---- per x-tensor: d-reduce (max/min), bf16 cast, row shuffle ----
        # x shard viewed as [128, 1024]: partition p = b*16 + n//64,
        # free f = (n%64)*16 + d.
        xt = [xt0, xt1]
        from concourse.bass import _add_dep_helper

        r_t = []
        last_cast = None
        for xi in range(2):
            x3 = xt[xi].rearrange("p (c d) -> p c d", d=D)
            xmax = spool.tile([128, 64], F32, tag=f"xmax{xi}", name="xmax")
            xmin = spool.tile([128, 64], F32, tag=f"xmin{xi}", name="xmin")
            comb = spool.tile([128, 128], BF16, tag=f"comb{xi}", name="comb")
            # DVE stream: red_max, cast_max, red_min, cast_min - each cast
            # unblocks its row-shuffle DMAs as early as possible
            ops = []
            ops.append(nc.vector.tensor_reduce(xmax[:], x3, axis=AX.X, op=ALU.max))
            ops.append(nc.vector.tensor_copy(comb[:, 0:64], xmax[:]))
            ops.append(nc.vector.tensor_reduce(xmin[:], x3, axis=AX.X, op=ALU.min))
            ops.append(nc.vector.tensor_copy(comb[:, 64:128], xmin[:]))
            # cast_max must precede red_min in the static DVE order so the
            # xm row shuffles launch while red_min still runs
            _add_dep_helper(
                ops[2].ins, ops[1].ins, sync=False, reason="cast before next reduce"
            )
            if last_cast is not None:
                # keep x1's DVE work behind x0's casts in the static DVE
                # stream (the scheduler otherwise interleaves them and
                # head-of-line-blocks x0's row shuffle for ~2us)
                for op in ops:
                    _add_dep_helper(
                        op.ins, last_cast.ins, sync=False, reason="x0 chain first"
                    )
            last_cast = ops[-1]
            if xi == 0:
                r_ops0 = ops[0]
            # R rows = xm xm xn xn; free = b*1024 + c*64 + j (n = c*64+j).
            # Row transfers are descriptor-bound (~20ns x 16 descriptors
            # per batch), so x0's rows are split into an early b0-2 chunk
            # (lets the matmul/sigmoid phase start ~2us sooner) + bulk.
            rt = spool.tile(
                [66 if xi == 0 else 4, B_LOC * N], BF16, tag=f"r{xi}", name="rt"
            )
            if xi == 0:
                chunks = [(0, 3), (3, B_LOC)]
                engs = [nc.sync, nc.scalar]
            else:
                chunks = [(0, B_LOC)]
                engs = [nc.sync]
            for ci, (b0, b1) in enumerate(chunks):
                for row, src in enumerate([0, 0, 1, 1]):
                    engs[row % len(engs)].dma_start(
                        rt[
                            row : row + 1, b0 * N : b1 * N
                        ].rearrange("p (b c j) -> p b c j", c=16, j=64),
                        comb[b0 * 16 : b1 * 16, src * 64 : (src + 1) * 64],
                    )
                if xi == 0 and ci == 0:
                    # xn rows for batch 0 replicated at partitions 64-65
                    for k in range(2):
                        engs[k].dma_start(
                            rt[64 + k : 65 + k, 0:N].rearrange(
                                "p (b c j) -> p b c j", c=16, j=64
                            ),
                            comb[0:16, 64:128],
                        )
            r_t.append(rt)

        # ---- branch core: K=4 matmul + fused sigmoid/bias/accum ----
        # lhs rows: wph wpl wnh wnl pair with rt rows: xm xm xn xn
        def unit(br, b):
            lt = lhs[0:4, br * M : (br + 1) * M]
            vt = vv[:, br : br + 1]
            ps = pspool.tile([M, N], F32, tag="s", name="ps")
            rhs = r_t[br][0:4, b * N : (b + 1) * N]
            nc.tensor.matmul(ps[:, 0:512], lt, rhs[:, 0:512], start=True, stop=True)
            nc.tensor.matmul(
                ps[:, 512:1024], lt, rhs[:, 512:1024], start=True, stop=True
            )
            # in-place PSUM write: the elementwise output is dead (only
            # accum_out is consumed) and PSUM access is cheaper than SBUF
            nc.scalar.activation(
                ps[:],
                ps[:],
                AF.Sigmoid,
                bias=vt,
                accum_out=z_t[br][:, b : b + 1],
            )

        # ---- MLP head (per batch-half so half 0 hides under sigmoids):
        # h = sigmoid(W_h @ z); y = C_w @ h + C_b ----
        y_half = [
            spool.tile([4, O], F32, tag=f"ysb{h}", name=f"ysb{h}")
            for h in range(2)
        ]

        def mlp_half(half):
            bs = slice(half * 4, half * 4 + 4)
            h_t = []
            for hh in range(2):
                hp = mlppool.tile([128, 4], F32, tag="mlp", name="hp")
                nc.tensor.matmul(
                    hp[:],
                    pack[:, PK_WHT0 + hh * 128 : PK_WHT0 + (hh + 1) * 128],
                    z_t[0][:, bs],
                    start=True,
                    stop=False,
                )
                nc.tensor.matmul(
                    hp[:],
                    pack[:, PK_WHT1 + hh * 128 : PK_WHT1 + (hh + 1) * 128],
                    z_t[1][:, bs],
                    start=False,
                    stop=True,
                )
                hs = spool.tile([128, 4], F32, tag=f"hs{hh}_{half}", name="hs")
                nc.scalar.activation(hs[:], hp[:], AF.Sigmoid)
                h_t.append(hs)
            yp = mlppool.tile([4, O], F32, tag="mlp", name="yp")
            nc.tensor.matmul(
                yp[:], h_t[0][:], pack[:, PK_CW0 : PK_CW0 + O], start=True, stop=False
            )
            nc.tensor.matmul(
                yp[:], h_t[1][:], pack[:, PK_CW1 : PK_CW1 + O], start=False, stop=False
            )
            nc.tensor.matmul(yp[:], ones_t[:, 0:4], cb_t, start=False, stop=True)
            nc.vector.tensor_copy(y_half[half][:], yp[:])

        for b in range(B_LOC):
            unit(0, b)
        for b in range(B_LOC):
            unit(1, b)
            if b == 3:
                mlp_half(0)
        mlp_half(1)
        nc.sync.dma_start(io["y"][0:4, :], y_half[0][:])
        nc.scalar.dma_start(io["y"][4:8, :], y_half[1][:])


_CACHED = None


def _build():
    global _CACHED
    if _CACHED is not None:
        return _CACHED
    nc = bacc.Bacc(
        "TRN2", target_bir_lowering=False, debug=False, num_devices=NCORES
    )
    io = {}
    io["x0"] = nc.dram_tensor("x0", [128, N], F32, kind="ExternalInput").ap()
    io["x1p"] = nc.dram_tensor(
        "x1p", [128, X1P_COLS], F32, kind="ExternalInput"
    ).ap()
    io["lhs"] = nc.dram_tensor("lhs", [66, 2 * M], BF16, kind="ExternalInput").ap()
    io["cb"] = nc.dram_tensor("cb", [1, O], F32, kind="ExternalInput").ap()
    io["vv"] = nc.dram_tensor("vv", [128, 2], F32, kind="ExternalInput").ap()
    io["y"] = nc.dram_tensor("y", [B_LOC, O], F32, kind="ExternalOutput").ap()

    with tile.TileContext(nc) as tc:
        _emit(tc, io)
    nc.compile()
    _CACHED = nc
    return nc


def _prep_params(inputs):
    import ml_dtypes

    f = np.float32
    bf = ml_dtypes.bfloat16

    def branch_lhs(P, U, A):
        W_g = (U @ A).reshape(M, L).astype(np.float64)
        w = (W_g @ P.astype(np.float64))[:, 0]
        rows = []
        for part in (np.maximum(w, 0.0), np.minimum(w, 0.0)):
            hi = part.astype(f).astype(bf)
            lo = (part.astype(f) - hi.astype(f)).astype(bf)
            rows += [hi, lo]
        # rows: wph wpl wnh wnl (pair with R's xm xm xn xn)
        return np.stack(rows).astype(bf)

    pack = np.zeros((128, X1P_COLS - PK_BASE), dtype=f)
    pack[:, PK_WHT0 - PK_BASE : PK_WHT0 - PK_BASE + 256] = inputs["W_h"].T[0:128, :]
    pack[:, PK_WHT1 - PK_BASE : PK_WHT1 - PK_BASE + 256] = inputs["W_h"].T[128:256, :]
    pack[:, PK_CW0 - PK_BASE : PK_CW0 - PK_BASE + O] = inputs["C_w"].T[0:128, :]
    pack[:, PK_CW1 - PK_BASE : PK_CW1 - PK_BASE + O] = inputs["C_w"].T[128:256, :]
    pack[:, PK_V0 - PK_BASE] = inputs["V0"].astype(f)
    pack[:, PK_V1 - PK_BASE] = inputs["V1"].astype(f)

    lhs = np.concatenate(
        [
            branch_lhs(inputs["P0"], inputs["U0"], inputs["A0"]),
            branch_lhs(inputs["P1"], inputs["U1"], inputs["A1"]),
        ],
        axis=1,
    )  # [4, 256] bf16
    lhs_full = np.zeros((66, 2 * M), dtype=bf)
    lhs_full[0:4, :] = lhs
    lhs_full[64:66, 0:M] = lhs[2:4, 0:M]  # branch-0 (wnh, wnl) replica

    vv = np.stack([inputs["V0"].astype(f), inputs["V1"].astype(f)], axis=1)
    return {
        "pack": pack,
        "lhs": np.ascontiguousarray(lhs_full),
        "cb": np.ascontiguousarray(inputs["C_b"].reshape(1, O), dtype=f),
        "vv": np.ascontiguousarray(vv),
    }


def run(inputs, trace=False, **kw):
    nc = _build()
    params = _prep_params(inputs)
    x0 = np.ascontiguousarray(inputs["x0"], dtype=np.float32)
    x1 = np.ascontiguousarray(inputs["x1"], dtype=np.float32)
    in_maps = []
    for c in range(NCORES):
        m = {k: v for k, v in params.items() if k != "pack"}
        m["x0"] = x0[c * B_LOC : (c + 1) * B_LOC].reshape(128, N)
        m["x1p"] = np.concatenate(
            [x1[c * B_LOC : (c + 1) * B_LOC].reshape(128, N), params["pack"]],
            axis=1,
        )
        in_maps.append(m)
    res = run_bass_kernel_spmd(nc, in_maps, list(range(NCORES)), trace=trace, **kw)
    y = np.concatenate([res.results[c]["y"] for c in range(NCORES)], axis=0)
    return y, res


def kernel(**inputs):
    y, _ = run(inputs, trace=False)
    return y
